# revision 5
# baseline (speedup 1.0000x reference)
"""Trainium2 Bass kernel for EquivariantProductBasisBlock.

Strategy
--------
The per-node compute is feature-diagonal except for three matmuls that
contract over the feature axis (species-conditioned gate + two equivariant
linears).  We therefore keep F=128 on SBUF partitions and nodes on the free
axis, so every matmul is a natural PE op and every elementwise op is a
[128, W] streaming op.

The species-conditioned weights force species-uniform node tiles, so the
host sorts nodes by species and assigns 8 species per core (snake order on
descending counts for load balance).  Each core's program processes 8
"slots" (species segments) of identical padded width W = 128 * ceil(max
species count / 128) -- the program is identical across cores (SPMD); only
the data (which species lives in which slot) differs.

Per slot (all tiles [128, W] fp32):
    x0, a, b, c = x1 components            (DMA, host pre-transposed)
    ACT : x0sq/asq/bsq/csq = squares, B = w04*x0 + w02 (per-partition APs)
    POOL: dot = asq + bsq + csq
    DVE : p0 = w03*x0sq + w00 ; A = w01*x0 + p0
          t1 = x0*A ; t2 = dot*B          POOL: out0 = t1 + t2
    PE  : graw[h] = gk_s[:,h*128:+128].T @ out0        (h = 0, 1)
    DVE : o0g = (graw0 + gb0) * out0      (fused scalar_tensor_tensor)
          g0 = u1*x0 + u0 ; g1 = u2*x0sq + g0 ; G1 = u3*dot + g1
          H = (graw1 + gb1) * G1
          q_d = x1d * H                    (q0 on POOL)
    PE  : y0 = LW0.T @ o0g ; y1_d = LW1.T @ q_d        (LW = lin_w * inv)
    ACT : PSUM -> SBUF copies, then DMA out.

Zero padding is safe end-to-end (pad columns produce exact zeros).
"""

import numpy as np

F = 128
S = 64
NCORES = 8
NSLOTS = S // NCORES  # species slots per core
NP0 = 5
NP1 = 4

_PROG_CACHE = {}


def _plan(species):
    """Assign species to (core, slot) and compute padded slot width."""
    counts = np.bincount(species, minlength=S)
    order = np.argsort(-counts, kind="stable")
    core_slots = [[] for _ in range(NCORES)]
    for r in range(NSLOTS):
        cores = range(NCORES) if r % 2 == 0 else range(NCORES - 1, -1, -1)
        for i, c in enumerate(cores):
            core_slots[c].append(int(order[r * NCORES + i]))
    t_seg = max(1, -(-int(counts.max()) // 128))
    return core_slots, 128 * t_seg, counts


def _build_program(W):
    from contextlib import ExitStack

    import concourse.tile as tile
    from concourse import bacc, mybir

    f32 = mybir.dt.float32
    Alu = mybir.AluOpType
    Act = mybir.ActivationFunctionType
    R = NSLOTS * W
    nch = -(-W // 512)  # psum chunks per slot
    PW = 512 * nch      # psum tile width (bank aligned chunks)

    nc = bacc.Bacc(
        "TRN2", target_bir_lowering=False, debug=False, num_devices=NCORES
    )
    X = nc.dram_tensor("X", [4, F, R], f32, kind="ExternalInput").ap()
    W0T = nc.dram_tensor("W0T", [F, NSLOTS * NP0], f32, kind="ExternalInput").ap()
    W1T = nc.dram_tensor("W1T", [F, NSLOTS * NP1], f32, kind="ExternalInput").ap()
    GK = nc.dram_tensor("GK", [F, NSLOTS * 256], f32, kind="ExternalInput").ap()
    GB0 = nc.dram_tensor("GB0", [F, NSLOTS], f32, kind="ExternalInput").ap()
    GB1 = nc.dram_tensor("GB1", [F, NSLOTS], f32, kind="ExternalInput").ap()
    LW0 = nc.dram_tensor("LW0", [F, F], f32, kind="ExternalInput").ap()
    LW1 = nc.dram_tensor("LW1", [F, F], f32, kind="ExternalInput").ap()
    Y = nc.dram_tensor("Y", [4, F, R], f32, kind="ExternalOutput").ap()

    with tile.TileContext(nc) as tc:
        with ExitStack() as ctx:
            wp = ctx.enter_context(tc.tile_pool(name="w", bufs=1))
            inp = ctx.enter_context(tc.tile_pool(name="in", bufs=2))
            mid = ctx.enter_context(tc.tile_pool(name="mid", bufs=2))
            outp = ctx.enter_context(tc.tile_pool(name="out", bufs=2))
            ps = ctx.enter_context(tc.tile_pool(name="ps", bufs=4, space="PSUM"))

            w0t = wp.tile([F, NSLOTS * NP0], f32)
            w1t = wp.tile([F, NSLOTS * NP1], f32)
            gks = wp.tile([F, NSLOTS * 256], f32)
            gb0 = wp.tile([F, NSLOTS], f32)
            gb1 = wp.tile([F, NSLOTS], f32)
            lw0 = wp.tile([F, F], f32)
            lw1 = wp.tile([F, F], f32)
            nc.sync.dma_start(out=w0t[:], in_=W0T[:])
            nc.sync.dma_start(out=w1t[:], in_=W1T[:])
            nc.sync.dma_start(out=gks[:], in_=GK[:])
            nc.sync.dma_start(out=gb0[:], in_=GB0[:])
            nc.sync.dma_start(out=gb1[:], in_=GB1[:])
            nc.sync.dma_start(out=lw0[:], in_=LW0[:])
            nc.sync.dma_start(out=lw1[:], in_=LW1[:])

            def chunks():
                for c in range(nch):
                    lo = c * 512
                    yield slice(lo, min(W, lo + 512))

            for j in range(NSLOTS):
                sl = slice(j * W, (j + 1) * W)
                x0 = inp.tile([F, W], f32, tag="x0")
                xa = inp.tile([F, W], f32, tag="xa")
                xb = inp.tile([F, W], f32, tag="xb")
                xc = inp.tile([F, W], f32, tag="xc")
                nc.sync.dma_start(out=x0[:], in_=X[0, :, sl])
                nc.sync.dma_start(out=xa[:], in_=X[1, :, sl])
                nc.sync.dma_start(out=xb[:], in_=X[2, :, sl])
                nc.sync.dma_start(out=xc[:], in_=X[3, :, sl])

                # per-partition scalar views for this slot's species
                w00 = w0t[:, j * NP0 + 0 : j * NP0 + 1]
                w01 = w0t[:, j * NP0 + 1 : j * NP0 + 2]
                w02 = w0t[:, j * NP0 + 2 : j * NP0 + 3]
                w03 = w0t[:, j * NP0 + 3 : j * NP0 + 4]
                w04 = w0t[:, j * NP0 + 4 : j * NP0 + 5]
                u0 = w1t[:, j * NP1 + 0 : j * NP1 + 1]
                u1 = w1t[:, j * NP1 + 1 : j * NP1 + 2]
                u2 = w1t[:, j * NP1 + 2 : j * NP1 + 3]
                u3 = w1t[:, j * NP1 + 3 : j * NP1 + 4]
                b0 = gb0[:, j : j + 1]
                b1 = gb1[:, j : j + 1]

                x0sq = mid.tile([F, W], f32, tag="x0sq")
                asq = mid.tile([F, W], f32, tag="asq")
                bsq = mid.tile([F, W], f32, tag="bsq")
                csq = mid.tile([F, W], f32, tag="csq")
                nc.scalar.activation(x0sq[:], x0[:], Act.Square)
                nc.scalar.activation(asq[:], xa[:], Act.Square)
                nc.scalar.activation(bsq[:], xb[:], Act.Square)
                nc.scalar.activation(csq[:], xc[:], Act.Square)

                dot = mid.tile([F, W], f32, tag="dot")
                nc.gpsimd.tensor_tensor(dot[:], asq[:], bsq[:], Alu.add)
                nc.gpsimd.tensor_tensor(dot[:], dot[:], csq[:], Alu.add)

                p0 = mid.tile([F, W], f32, tag="p0")
                nc.vector.tensor_scalar(p0[:], x0sq[:], w03, w00, Alu.mult, Alu.add)
                aa = mid.tile([F, W], f32, tag="aa")
                nc.vector.scalar_tensor_tensor(
                    aa[:], x0[:], w01, p0[:], Alu.mult, Alu.add
                )
                bb = mid.tile([F, W], f32, tag="bb")
                nc.scalar.activation(bb[:], x0[:], Act.Identity, bias=w02, scale=w04)

                t1 = mid.tile([F, W], f32, tag="t1")
                nc.vector.tensor_tensor(t1[:], x0[:], aa[:], Alu.mult)
                t2 = mid.tile([F, W], f32, tag="t2")
                nc.vector.tensor_tensor(t2[:], dot[:], bb[:], Alu.mult)
                out0 = mid.tile([F, W], f32, tag="out0")
                nc.gpsimd.tensor_tensor(out0[:], t1[:], t2[:], Alu.add)

                # gate matmuls: graw[h] = gk[:, h-half].T @ out0
                praw0 = ps.tile([F, PW], f32, tag="ps")
                praw1 = ps.tile([F, PW], f32, tag="ps")
                for h, pr in ((0, praw0), (1, praw1)):
                    lhsT = gks[:, j * 256 + h * 128 : j * 256 + (h + 1) * 128]
                    for cs in chunks():
                        nc.tensor.matmul(
                            pr[:, cs], lhsT, out0[:, cs], start=True, stop=True
                        )

                g0 = mid.tile([F, W], f32, tag="g0")
                nc.vector.tensor_scalar(g0[:], x0[:], u1, u0, Alu.mult, Alu.add)
                g1 = mid.tile([F, W], f32, tag="g1")
                nc.vector.scalar_tensor_tensor(
                    g1[:], x0sq[:], u2, g0[:], Alu.mult, Alu.add
                )
                gg = mid.tile([F, W], f32, tag="gg")
                nc.vector.scalar_tensor_tensor(
                    gg[:], dot[:], u3, g1[:], Alu.mult, Alu.add
                )

                o0g = mid.tile([F, W], f32, tag="o0g")
                nc.vector.scalar_tensor_tensor(
                    o0g[:], praw0[:, :W], b0, out0[:], Alu.add, Alu.mult
                )
                hh = mid.tile([F, W], f32, tag="hh")
                nc.vector.scalar_tensor_tensor(
                    hh[:], praw1[:, :W], b1, gg[:], Alu.add, Alu.mult
                )

                q0 = mid.tile([F, W], f32, tag="q0")
                nc.gpsimd.tensor_tensor(q0[:], xa[:], hh[:], Alu.mult)
                q1 = mid.tile([F, W], f32, tag="q1")
                nc.vector.tensor_tensor(q1[:], xb[:], hh[:], Alu.mult)
                q2 = mid.tile([F, W], f32, tag="q2")
                nc.gpsimd.tensor_tensor(q2[:], xc[:], hh[:], Alu.mult)

                py = [
                    ps.tile([F, PW], f32, tag="ps", name=f"py{i}") for i in range(4)
                ]
                for t, rhs, lhsT in (
                    (py[0], o0g, lw0),
                    (py[1], q0, lw1),
                    (py[2], q1, lw1),
                    (py[3], q2, lw1),
                ):
                    for cs in chunks():
                        nc.tensor.matmul(
                            t[:, cs], lhsT[:], rhs[:, cs], start=True, stop=True
                        )

                for comp in range(4):
                    yc = outp.tile([F, W], f32, tag=f"yc{comp}")
                    nc.scalar.copy(yc[:], py[comp][:, :W])
                    nc.sync.dma_start(out=Y[comp, :, sl], in_=yc[:])

    nc.compile()
    return nc


def _get_program(W):
    if W not in _PROG_CACHE:
        _PROG_CACHE[W] = _build_program(W)
    return _PROG_CACHE[W]


def _prepare(node_feats, node_species, w0, w1, gate_kernel, gate_bias, lin_w0, lin_w1):
    N = node_feats.shape[0]
    species = np.asarray(node_species).astype(np.int64)
    core_slots, W, counts = _plan(species)
    R = NSLOTS * W

    perm = np.argsort(species, kind="stable")
    starts = np.zeros(S + 1, np.int64)
    starts[1:] = np.cumsum(counts)

    nf = np.ascontiguousarray(np.asarray(node_feats, dtype=np.float32))
    x0t = nf[:, :F].T  # [F, N]
    x1 = nf[:, F:].reshape(N, F, 3)
    xt = [x0t, x1[:, :, 0].T, x1[:, :, 1].T, x1[:, :, 2].T]

    inv = np.float32(1.0 / np.sqrt(F))
    lw0 = np.ascontiguousarray((np.asarray(lin_w0, np.float32) * inv))
    lw1 = np.ascontiguousarray((np.asarray(lin_w1, np.float32) * inv))
    w0 = np.asarray(w0, np.float32)
    w1 = np.asarray(w1, np.float32)
    gk = np.asarray(gate_kernel, np.float32)
    gb = np.asarray(gate_bias, np.float32)

    in_maps = []
    gathers = []  # (src_node_idx, dst_cols) per core
    for c in range(NCORES):
        slots = core_slots[c]
        src = []
        dst = []
        for j, s in enumerate(slots):
            n_s = int(counts[s])
            if n_s:
                src.append(perm[starts[s] : starts[s] + n_s])
                dst.append(np.arange(j * W, j * W + n_s, dtype=np.int64))
        src = np.concatenate(src) if src else np.zeros(0, np.int64)
        dst = np.concatenate(dst) if dst else np.zeros(0, np.int64)
        gathers.append((src, dst))

        Xc = np.zeros((4, F, R), np.float32)
        for comp in range(4):
            Xc[comp][:, dst] = xt[comp][:, src]

        sw0 = w0[slots]  # [8, 5, F]
        sw1 = w1[slots]
        in_maps.append(
            dict(
                X=Xc,
                W0T=np.ascontiguousarray(sw0.transpose(2, 0, 1).reshape(F, -1)),
                W1T=np.ascontiguousarray(sw1.transpose(2, 0, 1).reshape(F, -1)),
                GK=np.ascontiguousarray(gk[slots].transpose(1, 0, 2).reshape(F, -1)),
                GB0=np.ascontiguousarray(gb[slots][:, :F].T),
                GB1=np.ascontiguousarray(gb[slots][:, F:].T),
                LW0=lw0,
                LW1=lw1,
            )
        )
    return in_maps, gathers, W, N


def _assemble(results, gathers, N):
    out = np.empty((N, 4 * F), np.float32)
    y1t = np.empty((3, F, N), np.float32)
    y0t = np.empty((F, N), np.float32)
    for c in range(NCORES):
        Yc = results[c]["Y"]
        src, dst = gathers[c]
        y0t[:, src] = Yc[0][:, dst]
        for d in range(3):
            y1t[d][:, src] = Yc[1 + d][:, dst]
    out[:, :F] = y0t.T
    out[:, F:] = y1t.transpose(2, 1, 0).reshape(N, 3 * F)
    return out


def kernel(**inputs):
    from concourse.bass_utils import run_bass_kernel_spmd

    in_maps, gathers, W, N = _prepare(**inputs)
    nc = _get_program(W)
    res = run_bass_kernel_spmd(nc, in_maps, list(range(NCORES)))
    return _assemble(res.results, gathers, N)


# revision 8
# speedup vs baseline: 1.0987x; 1.0987x over previous
"""Trainium2 Bass kernel for EquivariantProductBasisBlock.

Strategy
--------
The per-node compute is feature-diagonal except for three matmuls that
contract over the feature axis (species-conditioned gate + two equivariant
linears).  We therefore keep F=128 on SBUF partitions and nodes on the free
axis, so every matmul is a natural PE op and every elementwise op is a
[128, W] streaming op.

The species-conditioned weights force species-uniform node tiles, so the
host sorts nodes by species and assigns 8 species per core (snake order on
descending counts for load balance).  Each core's program processes 8
"slots" (species segments) of identical padded width W = 128 * ceil(max
species count / 128) -- the program is identical across cores (SPMD); only
the data (which species lives in which slot) differs.

Device data layout: one fused input array X[f, slot*(4W) + comp*W + col]
(comp 0 = x0, 1..3 = x1 d-components, host pre-transposed) -> one big DMA
per slot; output Y mirrors it.  Matmuls run as float32r (same bits,
single-pass PE).  Per slot (tiles [128, W] fp32):

    ACT : x0sq/asq/bsq/csq = squares, A1 = w01*x0 + w00, g0 = u1*x0 + u0,
          B = w04*x0 + w02  (per-partition scale/bias APs)
    POOL: dot = asq + bsq + csq ; out0 = t1 + t2 ; q0 = a*H
    DVE : A = w03*x0sq + A1 ; t1 = x0*A ; t2 = dot*B
          g1 = u2*x0sq + g0 ; G1 = u3*dot + g1
    PE  : graw[h] = gk_s[:, h*128:+128].T @ out0        (h = 0, 1)
    DVE : o0g = (graw0 + gb0) * out0 ; H = (graw1 + gb1) * G1   (fused)
          q1 = b*H ; q2 = c*H
    PE  : y0 = LW0.T @ o0g ; y1_d = LW1.T @ q_d         (LW = lin_w * inv)
    ACT : PSUM -> yout slices, then one DMA out.

Zero padding is safe end-to-end (pad columns produce exact zeros).
"""

import numpy as np

F = 128
S = 64
NCORES = 8
NSLOTS = S // NCORES  # species slots per core
NP0 = 5
NP1 = 4

_PROG_CACHE = {}


def _plan(species):
    """Assign species to (core, slot) and compute padded slot width."""
    counts = np.bincount(species, minlength=S)
    order = np.argsort(-counts, kind="stable")
    core_slots = [[] for _ in range(NCORES)]
    for r in range(NSLOTS):
        cores = range(NCORES) if r % 2 == 0 else range(NCORES - 1, -1, -1)
        for i, c in enumerate(cores):
            core_slots[c].append(int(order[r * NCORES + i]))
    t_seg = max(1, -(-int(counts.max()) // 128))
    return core_slots, 128 * t_seg, counts


def _build_program(W):
    from contextlib import ExitStack

    import concourse.tile as tile
    from concourse import bacc, mybir

    f32 = mybir.dt.float32
    f32r = mybir.dt.float32r
    Alu = mybir.AluOpType
    Act = mybir.ActivationFunctionType
    R = NSLOTS * 4 * W  # fused layout: slot-major, comp, col
    nch = -(-W // 512)  # psum chunks per slot
    PW = 512 * nch      # psum tile width (bank aligned chunks)

    nc = bacc.Bacc(
        "TRN2", target_bir_lowering=False, debug=False, num_devices=NCORES
    )
    X = nc.dram_tensor("X", [F, R], f32, kind="ExternalInput").ap()
    W0T = nc.dram_tensor("W0T", [F, NSLOTS * NP0], f32, kind="ExternalInput").ap()
    W1T = nc.dram_tensor("W1T", [F, NSLOTS * NP1], f32, kind="ExternalInput").ap()
    GK = nc.dram_tensor("GK", [F, NSLOTS * 256], f32r, kind="ExternalInput").ap()
    GB0 = nc.dram_tensor("GB0", [F, NSLOTS], f32, kind="ExternalInput").ap()
    GB1 = nc.dram_tensor("GB1", [F, NSLOTS], f32, kind="ExternalInput").ap()
    LW0 = nc.dram_tensor("LW0", [F, F], f32r, kind="ExternalInput").ap()
    LW1 = nc.dram_tensor("LW1", [F, F], f32r, kind="ExternalInput").ap()
    Y = nc.dram_tensor("Y", [F, R], f32, kind="ExternalOutput").ap()

    def mm(psum_ap, lhsT, rhs, **kw):
        nc.tensor.matmul(psum_ap, lhsT, rhs, **kw)

    with tile.TileContext(nc) as tc:
        with ExitStack() as ctx:
            wp = ctx.enter_context(tc.tile_pool(name="w", bufs=1))
            inp = ctx.enter_context(tc.tile_pool(name="in", bufs=2))
            mid = ctx.enter_context(tc.tile_pool(name="mid", bufs=2))
            tmp = ctx.enter_context(tc.tile_pool(name="tmp", bufs=1))
            outp = ctx.enter_context(tc.tile_pool(name="out", bufs=2))
            ps = ctx.enter_context(tc.tile_pool(name="ps", bufs=4, space="PSUM"))

            w0t = wp.tile([F, NSLOTS * NP0], f32)
            w1t = wp.tile([F, NSLOTS * NP1], f32)
            gks = wp.tile([F, NSLOTS * 256], f32r)
            gb0 = wp.tile([F, NSLOTS], f32)
            gb1 = wp.tile([F, NSLOTS], f32)
            lw0 = wp.tile([F, F], f32r)
            lw1 = wp.tile([F, F], f32r)
            nc.sync.dma_start(out=w0t[:], in_=W0T[:])
            nc.sync.dma_start(out=w1t[:], in_=W1T[:])
            nc.sync.dma_start(out=gks[:], in_=GK[:])
            nc.sync.dma_start(out=gb0[:], in_=GB0[:])
            nc.sync.dma_start(out=gb1[:], in_=GB1[:])
            nc.sync.dma_start(out=lw0[:], in_=LW0[:])
            nc.sync.dma_start(out=lw1[:], in_=LW1[:])

            def chunks():
                for c in range(nch):
                    lo = c * 512
                    yield slice(lo, min(W, lo + 512))

            for j in range(NSLOTS):
                xin = inp.tile([F, 4 * W], f32, tag="xin")
                nc.sync.dma_start(
                    out=xin[:], in_=X[:, j * 4 * W : (j + 1) * 4 * W]
                )
                x0 = xin[:, 0:W]
                xa = xin[:, W : 2 * W]
                xb = xin[:, 2 * W : 3 * W]
                xc = xin[:, 3 * W : 4 * W]

                # per-partition scalar views for this slot's species
                w00 = w0t[:, j * NP0 + 0 : j * NP0 + 1]
                w01 = w0t[:, j * NP0 + 1 : j * NP0 + 2]
                w02 = w0t[:, j * NP0 + 2 : j * NP0 + 3]
                w03 = w0t[:, j * NP0 + 3 : j * NP0 + 4]
                w04 = w0t[:, j * NP0 + 4 : j * NP0 + 5]
                u0 = w1t[:, j * NP1 + 0 : j * NP1 + 1]
                u1 = w1t[:, j * NP1 + 1 : j * NP1 + 2]
                u2 = w1t[:, j * NP1 + 2 : j * NP1 + 3]
                u3 = w1t[:, j * NP1 + 3 : j * NP1 + 4]
                b0 = gb0[:, j : j + 1]
                b1 = gb1[:, j : j + 1]

                x0sq = mid.tile([F, W], f32, tag="x0sq")
                asq = tmp.tile([F, W], f32, tag="asq")
                bsq = tmp.tile([F, W], f32, tag="bsq")
                csq = tmp.tile([F, W], f32, tag="csq")
                nc.scalar.activation(x0sq[:], x0[:], Act.Square)
                nc.scalar.activation(asq[:], xa[:], Act.Square)
                nc.scalar.activation(bsq[:], xb[:], Act.Square)
                nc.scalar.activation(csq[:], xc[:], Act.Square)

                dot = mid.tile([F, W], f32, tag="dot")
                nc.gpsimd.tensor_tensor(dot[:], asq[:], bsq[:], Alu.add)
                nc.gpsimd.tensor_tensor(dot[:], dot[:], csq[:], Alu.add)

                a1 = tmp.tile([F, W], f32, tag="a1")
                nc.scalar.activation(a1[:], x0[:], Act.Identity, bias=w00, scale=w01)
                aa = mid.tile([F, W], f32, tag="aa")
                nc.vector.scalar_tensor_tensor(
                    aa[:], x0sq[:], w03, a1[:], Alu.mult, Alu.add
                )
                bb = tmp.tile([F, W], f32, tag="bb")
                nc.scalar.activation(bb[:], x0[:], Act.Identity, bias=w02, scale=w04)

                t1 = tmp.tile([F, W], f32, tag="t1")
                nc.vector.tensor_tensor(t1[:], x0[:], aa[:], Alu.mult)
                t2 = tmp.tile([F, W], f32, tag="t2")
                nc.vector.tensor_tensor(t2[:], dot[:], bb[:], Alu.mult)
                out0 = mid.tile([F, W], f32r, tag="out0")
                nc.gpsimd.tensor_tensor(out0[:], t1[:], t2[:], Alu.add)

                # gate matmuls: graw[h] = gk[:, h-half].T @ out0
                praw0 = ps.tile([F, PW], f32, tag="ps")
                praw1 = ps.tile([F, PW], f32, tag="ps")
                for h, pr in ((0, praw0), (1, praw1)):
                    lhsT = gks[:, j * 256 + h * 128 : j * 256 + (h + 1) * 128]
                    for cs in chunks():
                        mm(pr[:, cs], lhsT, out0[:, cs], start=True, stop=True)

                g0 = tmp.tile([F, W], f32, tag="g0")
                nc.scalar.activation(g0[:], x0[:], Act.Identity, bias=u0, scale=u1)
                g1 = tmp.tile([F, W], f32, tag="g1")
                nc.vector.scalar_tensor_tensor(
                    g1[:], x0sq[:], u2, g0[:], Alu.mult, Alu.add
                )
                gg = mid.tile([F, W], f32, tag="gg")
                nc.vector.scalar_tensor_tensor(
                    gg[:], dot[:], u3, g1[:], Alu.mult, Alu.add
                )

                o0g = mid.tile([F, W], f32r, tag="o0g")
                nc.vector.scalar_tensor_tensor(
                    o0g[:], praw0[:, :W], b0, out0[:], Alu.add, Alu.mult
                )
                hh = mid.tile([F, W], f32, tag="hh")
                nc.vector.scalar_tensor_tensor(
                    hh[:], praw1[:, :W], b1, gg[:], Alu.add, Alu.mult
                )

                q0 = mid.tile([F, W], f32r, tag="q0")
                nc.gpsimd.tensor_tensor(q0[:], xa[:], hh[:], Alu.mult)
                q1 = mid.tile([F, W], f32r, tag="q1")
                nc.vector.tensor_tensor(q1[:], xb[:], hh[:], Alu.mult)
                q2 = mid.tile([F, W], f32r, tag="q2")
                nc.vector.tensor_tensor(q2[:], xc[:], hh[:], Alu.mult)

                py = [
                    ps.tile([F, PW], f32, tag="ps", name=f"py{i}") for i in range(4)
                ]
                for t, rhs, lhsT in (
                    (py[0], o0g, lw0),
                    (py[1], q0, lw1),
                    (py[2], q1, lw1),
                    (py[3], q2, lw1),
                ):
                    for cs in chunks():
                        mm(t[:, cs], lhsT[:], rhs[:, cs], start=True, stop=True)

                yout = outp.tile([F, 4 * W], f32, tag="yout")
                for comp in range(4):
                    nc.scalar.copy(
                        yout[:, comp * W : (comp + 1) * W], py[comp][:, :W]
                    )
                nc.sync.dma_start(
                    out=Y[:, j * 4 * W : (j + 1) * 4 * W], in_=yout[:]
                )

    nc.compile()
    return nc


def _get_program(W):
    if W not in _PROG_CACHE:
        _PROG_CACHE[W] = _build_program(W)
    return _PROG_CACHE[W]


def _prepare(node_feats, node_species, w0, w1, gate_kernel, gate_bias, lin_w0, lin_w1):
    N = node_feats.shape[0]
    species = np.asarray(node_species).astype(np.int64)
    core_slots, W, counts = _plan(species)
    R = NSLOTS * 4 * W

    perm = np.argsort(species, kind="stable")
    starts = np.zeros(S + 1, np.int64)
    starts[1:] = np.cumsum(counts)

    nf = np.ascontiguousarray(np.asarray(node_feats, dtype=np.float32))
    x0t = nf[:, :F].T  # [F, N]
    x1 = nf[:, F:].reshape(N, F, 3)
    xt = [x0t, x1[:, :, 0].T, x1[:, :, 1].T, x1[:, :, 2].T]

    inv = np.float32(1.0 / np.sqrt(F))
    lw0 = np.ascontiguousarray(np.asarray(lin_w0, np.float32) * inv)
    lw1 = np.ascontiguousarray(np.asarray(lin_w1, np.float32) * inv)
    w0 = np.asarray(w0, np.float32)
    w1 = np.asarray(w1, np.float32)
    gk = np.asarray(gate_kernel, np.float32)
    gb = np.asarray(gate_bias, np.float32)

    in_maps = []
    gathers = []  # per core: (src_node_idx, per-comp dst cols)
    for c in range(NCORES):
        slots = core_slots[c]
        src = []
        base = []  # col position of each node within its slot (j*4W + col)
        for j, s in enumerate(slots):
            n_s = int(counts[s])
            if n_s:
                src.append(perm[starts[s] : starts[s] + n_s])
                base.append(np.arange(n_s, dtype=np.int64) + j * 4 * W)
        src = np.concatenate(src) if src else np.zeros(0, np.int64)
        base = np.concatenate(base) if base else np.zeros(0, np.int64)
        gathers.append((src, base))

        Xc = np.zeros((F, R), np.float32)
        for comp in range(4):
            Xc[:, base + comp * W] = xt[comp][:, src]

        sw0 = w0[slots]  # [8, 5, F]
        sw1 = w1[slots]
        in_maps.append(
            dict(
                X=Xc,
                W0T=np.ascontiguousarray(sw0.transpose(2, 0, 1).reshape(F, -1)),
                W1T=np.ascontiguousarray(sw1.transpose(2, 0, 1).reshape(F, -1)),
                GK=np.ascontiguousarray(gk[slots].transpose(1, 0, 2).reshape(F, -1)),
                GB0=np.ascontiguousarray(gb[slots][:, :F].T),
                GB1=np.ascontiguousarray(gb[slots][:, F:].T),
                LW0=lw0,
                LW1=lw1,
            )
        )
    return in_maps, gathers, W, N


def _assemble(results, gathers, W, N):
    out = np.empty((N, 4 * F), np.float32)
    y0t = np.empty((F, N), np.float32)
    y1t = np.empty((3, F, N), np.float32)
    for c in range(NCORES):
        Yc = results[c]["Y"]
        src, base = gathers[c]
        y0t[:, src] = Yc[:, base]
        for d in range(3):
            y1t[d][:, src] = Yc[:, base + (1 + d) * W]
    out[:, :F] = y0t.T
    out[:, F:] = y1t.transpose(2, 1, 0).reshape(N, 3 * F)
    return out


def kernel(**inputs):
    from concourse.bass_utils import run_bass_kernel_spmd

    in_maps, gathers, W, N = _prepare(**inputs)
    nc = _get_program(W)
    res = run_bass_kernel_spmd(nc, in_maps, list(range(NCORES)))
    return _assemble(res.results, gathers, W, N)


# revision 9
# speedup vs baseline: 1.1773x; 1.0716x over previous
"""Trainium2 Bass kernel for EquivariantProductBasisBlock.

Strategy
--------
The per-node compute is feature-diagonal except for three matmuls that
contract over the feature axis (species-conditioned gate + two equivariant
linears).  We therefore keep F=128 on SBUF partitions and nodes on the free
axis, so every matmul is a natural PE op and every elementwise op is a
[128, W] streaming op.

The species-conditioned weights force species-uniform node tiles, so the
host sorts nodes by species and assigns 8 species per core (snake order on
descending counts for load balance).  Each core's program processes 8
"slots" (species segments) of identical padded width W = 128 * ceil(max
species count / 128) -- the program is identical across cores (SPMD); only
the data (which species lives in which slot) differs.

Device data layout: one fused input array X[f, slot*(4W) + comp*W + col]
(comp 0 = x0, 1..3 = x1 d-components, host pre-transposed) -> one big DMA
per slot; output Y mirrors it.  Matmuls run as float32r (same bits,
single-pass PE).  Per slot (tiles [128, W] fp32):

    ACT : x0sq/asq/bsq/csq = squares, A1 = w01*x0 + w00, g0 = u1*x0 + u0,
          B = w04*x0 + w02  (per-partition scale/bias APs)
    POOL: dot = asq + bsq + csq ; out0 = t1 + t2 ; q0 = a*H
    DVE : A = w03*x0sq + A1 ; t1 = x0*A ; t2 = dot*B
          g1 = u2*x0sq + g0 ; G1 = u3*dot + g1
    PE  : graw[h] = gk_s[:, h*128:+128].T @ out0        (h = 0, 1)
    DVE : o0g = (graw0 + gb0) * out0 ; H = (graw1 + gb1) * G1   (fused)
          q1 = b*H ; q2 = c*H
    PE  : y0 = LW0.T @ o0g ; y1_d = LW1.T @ q_d         (LW = lin_w * inv)
    ACT : PSUM -> yout slices, then one DMA out.

Zero padding is safe end-to-end (pad columns produce exact zeros).
"""

import numpy as np

F = 128
S = 64
NCORES = 8
NSLOTS = S // NCORES  # species slots per core
NP0 = 5
NP1 = 4

_PROG_CACHE = {}


def _plan(species):
    """Assign species to (core, slot) and compute padded slot width."""
    counts = np.bincount(species, minlength=S)
    order = np.argsort(-counts, kind="stable")
    core_slots = [[] for _ in range(NCORES)]
    for r in range(NSLOTS):
        cores = range(NCORES) if r % 2 == 0 else range(NCORES - 1, -1, -1)
        for i, c in enumerate(cores):
            core_slots[c].append(int(order[r * NCORES + i]))
    t_seg = max(1, -(-int(counts.max()) // 128))
    return core_slots, 128 * t_seg, counts


def _build_program(W):
    from contextlib import ExitStack

    import concourse.tile as tile
    from concourse import bacc, mybir

    f32 = mybir.dt.float32
    f32r = mybir.dt.float32r
    Alu = mybir.AluOpType
    Act = mybir.ActivationFunctionType
    R = NSLOTS * 4 * W  # fused layout: slot-major, comp, col
    nch = -(-W // 512)  # psum chunks per slot
    PW = 512 * nch      # psum tile width (bank aligned chunks)

    nc = bacc.Bacc(
        "TRN2", target_bir_lowering=False, debug=False, num_devices=NCORES
    )
    X = nc.dram_tensor("X", [F, R], f32, kind="ExternalInput").ap()
    W0T = nc.dram_tensor("W0T", [F, NSLOTS * NP0], f32, kind="ExternalInput").ap()
    W1T = nc.dram_tensor("W1T", [F, NSLOTS * NP1], f32, kind="ExternalInput").ap()
    GK = nc.dram_tensor("GK", [F, NSLOTS * 256], f32r, kind="ExternalInput").ap()
    GB0 = nc.dram_tensor("GB0", [F, NSLOTS], f32, kind="ExternalInput").ap()
    GB1 = nc.dram_tensor("GB1", [F, NSLOTS], f32, kind="ExternalInput").ap()
    LW0 = nc.dram_tensor("LW0", [F, F], f32r, kind="ExternalInput").ap()
    LW1 = nc.dram_tensor("LW1", [F, F], f32r, kind="ExternalInput").ap()
    Y = nc.dram_tensor("Y", [F, R], f32, kind="ExternalOutput").ap()

    def mm(psum_ap, lhsT, rhs, **kw):
        nc.tensor.matmul(psum_ap, lhsT, rhs, **kw)

    with tile.TileContext(nc) as tc:
        with ExitStack() as ctx:
            wp = ctx.enter_context(tc.tile_pool(name="w", bufs=1))
            inp = ctx.enter_context(tc.tile_pool(name="in", bufs=3))
            mid = ctx.enter_context(tc.tile_pool(name="mid", bufs=2))
            sqp = ctx.enter_context(tc.tile_pool(name="sq", bufs=6))
            acp = ctx.enter_context(tc.tile_pool(name="ac", bufs=4))
            ttp = ctx.enter_context(tc.tile_pool(name="tt", bufs=4))
            outp = ctx.enter_context(tc.tile_pool(name="out", bufs=2))
            ps = ctx.enter_context(tc.tile_pool(name="ps", bufs=4, space="PSUM"))

            w0t = wp.tile([F, NSLOTS * NP0], f32)
            w1t = wp.tile([F, NSLOTS * NP1], f32)
            gks = wp.tile([F, NSLOTS * 256], f32r)
            gb0 = wp.tile([F, NSLOTS], f32)
            gb1 = wp.tile([F, NSLOTS], f32)
            lw0 = wp.tile([F, F], f32r)
            lw1 = wp.tile([F, F], f32r)
            nc.sync.dma_start(out=w0t[:], in_=W0T[:])
            nc.sync.dma_start(out=w1t[:], in_=W1T[:])
            nc.sync.dma_start(out=gks[:], in_=GK[:])
            nc.sync.dma_start(out=gb0[:], in_=GB0[:])
            nc.sync.dma_start(out=gb1[:], in_=GB1[:])
            nc.sync.dma_start(out=lw0[:], in_=LW0[:])
            nc.sync.dma_start(out=lw1[:], in_=LW1[:])

            def chunks():
                for c in range(nch):
                    lo = c * 512
                    yield slice(lo, min(W, lo + 512))

            for j in range(NSLOTS):
                xin = inp.tile([F, 4 * W], f32, tag="xin")
                nc.sync.dma_start(
                    out=xin[:], in_=X[:, j * 4 * W : (j + 1) * 4 * W]
                )
                x0 = xin[:, 0:W]
                xa = xin[:, W : 2 * W]
                xb = xin[:, 2 * W : 3 * W]
                xc = xin[:, 3 * W : 4 * W]

                # per-partition scalar views for this slot's species
                w00 = w0t[:, j * NP0 + 0 : j * NP0 + 1]
                w01 = w0t[:, j * NP0 + 1 : j * NP0 + 2]
                w02 = w0t[:, j * NP0 + 2 : j * NP0 + 3]
                w03 = w0t[:, j * NP0 + 3 : j * NP0 + 4]
                w04 = w0t[:, j * NP0 + 4 : j * NP0 + 5]
                u0 = w1t[:, j * NP1 + 0 : j * NP1 + 1]
                u1 = w1t[:, j * NP1 + 1 : j * NP1 + 2]
                u2 = w1t[:, j * NP1 + 2 : j * NP1 + 3]
                u3 = w1t[:, j * NP1 + 3 : j * NP1 + 4]
                b0 = gb0[:, j : j + 1]
                b1 = gb1[:, j : j + 1]

                x0sq = sqp.tile([F, W], f32, tag="sq", name="x0sq")
                asq = sqp.tile([F, W], f32, tag="sq", name="asq")
                bsq = sqp.tile([F, W], f32, tag="sq", name="bsq")
                csq = sqp.tile([F, W], f32, tag="sq", name="csq")
                nc.scalar.activation(x0sq[:], x0[:], Act.Square)
                nc.scalar.activation(asq[:], xa[:], Act.Square)
                nc.scalar.activation(bsq[:], xb[:], Act.Square)
                nc.scalar.activation(csq[:], xc[:], Act.Square)

                dot = mid.tile([F, W], f32, tag="dot")
                nc.gpsimd.tensor_tensor(dot[:], asq[:], bsq[:], Alu.add)
                nc.gpsimd.tensor_tensor(dot[:], dot[:], csq[:], Alu.add)

                a1 = acp.tile([F, W], f32, tag="ac", name="a1")
                nc.scalar.activation(a1[:], x0[:], Act.Identity, bias=w00, scale=w01)
                aa = mid.tile([F, W], f32, tag="aa")
                nc.vector.scalar_tensor_tensor(
                    aa[:], x0sq[:], w03, a1[:], Alu.mult, Alu.add
                )
                bb = acp.tile([F, W], f32, tag="ac", name="bb")
                nc.scalar.activation(bb[:], x0[:], Act.Identity, bias=w02, scale=w04)

                t1 = ttp.tile([F, W], f32, tag="tt", name="t1")
                nc.vector.tensor_tensor(t1[:], x0[:], aa[:], Alu.mult)
                t2 = ttp.tile([F, W], f32, tag="tt", name="t2")
                nc.vector.tensor_tensor(t2[:], dot[:], bb[:], Alu.mult)
                out0 = mid.tile([F, W], f32r, tag="out0")
                nc.gpsimd.tensor_tensor(out0[:], t1[:], t2[:], Alu.add)

                # gate matmuls: graw[h] = gk[:, h-half].T @ out0
                praw0 = ps.tile([F, PW], f32, tag="ps")
                praw1 = ps.tile([F, PW], f32, tag="ps")
                for h, pr in ((0, praw0), (1, praw1)):
                    lhsT = gks[:, j * 256 + h * 128 : j * 256 + (h + 1) * 128]
                    for cs in chunks():
                        mm(pr[:, cs], lhsT, out0[:, cs], start=True, stop=True)

                g0 = acp.tile([F, W], f32, tag="ac", name="g0")
                nc.scalar.activation(g0[:], x0[:], Act.Identity, bias=u0, scale=u1)
                g1 = ttp.tile([F, W], f32, tag="tt", name="g1")
                nc.vector.scalar_tensor_tensor(
                    g1[:], x0sq[:], u2, g0[:], Alu.mult, Alu.add
                )
                gg = mid.tile([F, W], f32, tag="gg")
                nc.vector.scalar_tensor_tensor(
                    gg[:], dot[:], u3, g1[:], Alu.mult, Alu.add
                )

                o0g = mid.tile([F, W], f32r, tag="o0g")
                nc.vector.scalar_tensor_tensor(
                    o0g[:], praw0[:, :W], b0, out0[:], Alu.add, Alu.mult
                )
                hh = mid.tile([F, W], f32, tag="hh")
                nc.vector.scalar_tensor_tensor(
                    hh[:], praw1[:, :W], b1, gg[:], Alu.add, Alu.mult
                )

                q0 = mid.tile([F, W], f32r, tag="q0")
                nc.gpsimd.tensor_tensor(q0[:], xa[:], hh[:], Alu.mult)
                q1 = mid.tile([F, W], f32r, tag="q1")
                nc.vector.tensor_tensor(q1[:], xb[:], hh[:], Alu.mult)
                q2 = mid.tile([F, W], f32r, tag="q2")
                nc.vector.tensor_tensor(q2[:], xc[:], hh[:], Alu.mult)

                py = [
                    ps.tile([F, PW], f32, tag="ps", name=f"py{i}") for i in range(4)
                ]
                for t, rhs, lhsT in (
                    (py[0], o0g, lw0),
                    (py[1], q0, lw1),
                    (py[2], q1, lw1),
                    (py[3], q2, lw1),
                ):
                    for cs in chunks():
                        mm(t[:, cs], lhsT[:], rhs[:, cs], start=True, stop=True)

                yout = outp.tile([F, 4 * W], f32, tag="yout")
                for comp in range(4):
                    nc.scalar.copy(
                        yout[:, comp * W : (comp + 1) * W], py[comp][:, :W]
                    )
                nc.sync.dma_start(
                    out=Y[:, j * 4 * W : (j + 1) * 4 * W], in_=yout[:]
                )

    nc.compile()
    return nc


def _get_program(W):
    if W not in _PROG_CACHE:
        _PROG_CACHE[W] = _build_program(W)
    return _PROG_CACHE[W]


def _prepare(node_feats, node_species, w0, w1, gate_kernel, gate_bias, lin_w0, lin_w1):
    N = node_feats.shape[0]
    species = np.asarray(node_species).astype(np.int64)
    core_slots, W, counts = _plan(species)
    R = NSLOTS * 4 * W

    perm = np.argsort(species, kind="stable")
    starts = np.zeros(S + 1, np.int64)
    starts[1:] = np.cumsum(counts)

    nf = np.ascontiguousarray(np.asarray(node_feats, dtype=np.float32))
    x0t = nf[:, :F].T  # [F, N]
    x1 = nf[:, F:].reshape(N, F, 3)
    xt = [x0t, x1[:, :, 0].T, x1[:, :, 1].T, x1[:, :, 2].T]

    inv = np.float32(1.0 / np.sqrt(F))
    lw0 = np.ascontiguousarray(np.asarray(lin_w0, np.float32) * inv)
    lw1 = np.ascontiguousarray(np.asarray(lin_w1, np.float32) * inv)
    w0 = np.asarray(w0, np.float32)
    w1 = np.asarray(w1, np.float32)
    gk = np.asarray(gate_kernel, np.float32)
    gb = np.asarray(gate_bias, np.float32)

    in_maps = []
    gathers = []  # per core: (src_node_idx, per-comp dst cols)
    for c in range(NCORES):
        slots = core_slots[c]
        src = []
        base = []  # col position of each node within its slot (j*4W + col)
        for j, s in enumerate(slots):
            n_s = int(counts[s])
            if n_s:
                src.append(perm[starts[s] : starts[s] + n_s])
                base.append(np.arange(n_s, dtype=np.int64) + j * 4 * W)
        src = np.concatenate(src) if src else np.zeros(0, np.int64)
        base = np.concatenate(base) if base else np.zeros(0, np.int64)
        gathers.append((src, base))

        Xc = np.zeros((F, R), np.float32)
        for comp in range(4):
            Xc[:, base + comp * W] = xt[comp][:, src]

        sw0 = w0[slots]  # [8, 5, F]
        sw1 = w1[slots]
        in_maps.append(
            dict(
                X=Xc,
                W0T=np.ascontiguousarray(sw0.transpose(2, 0, 1).reshape(F, -1)),
                W1T=np.ascontiguousarray(sw1.transpose(2, 0, 1).reshape(F, -1)),
                GK=np.ascontiguousarray(gk[slots].transpose(1, 0, 2).reshape(F, -1)),
                GB0=np.ascontiguousarray(gb[slots][:, :F].T),
                GB1=np.ascontiguousarray(gb[slots][:, F:].T),
                LW0=lw0,
                LW1=lw1,
            )
        )
    return in_maps, gathers, W, N


def _assemble(results, gathers, W, N):
    out = np.empty((N, 4 * F), np.float32)
    y0t = np.empty((F, N), np.float32)
    y1t = np.empty((3, F, N), np.float32)
    for c in range(NCORES):
        Yc = results[c]["Y"]
        src, base = gathers[c]
        y0t[:, src] = Yc[:, base]
        for d in range(3):
            y1t[d][:, src] = Yc[:, base + (1 + d) * W]
    out[:, :F] = y0t.T
    out[:, F:] = y1t.transpose(2, 1, 0).reshape(N, 3 * F)
    return out


def kernel(**inputs):
    from concourse.bass_utils import run_bass_kernel_spmd

    in_maps, gathers, W, N = _prepare(**inputs)
    nc = _get_program(W)
    res = run_bass_kernel_spmd(nc, in_maps, list(range(NCORES)))
    return _assemble(res.results, gathers, W, N)


# revision 10
# speedup vs baseline: 1.3616x; 1.1565x over previous
"""Trainium2 Bass kernel for EquivariantProductBasisBlock.

Strategy
--------
The per-node compute is feature-diagonal except for three matmuls that
contract over the feature axis (species-conditioned gate + two equivariant
linears).  We therefore keep F=128 on SBUF partitions and nodes on the free
axis, so every matmul is a natural PE op and every elementwise op is a
[128, W] streaming op.

The species-conditioned weights force species-uniform node tiles, so the
host sorts nodes by species and assigns 8 species per core (snake order on
descending counts for load balance).  Each core's program processes 8
"slots" (species segments) of identical padded width W = 128 * ceil(max
species count / 128) -- the program is identical across cores (SPMD); only
the data (which species lives in which slot) differs.

Device data layout: one fused input array X[f, slot*(4W) + comp*W + col]
(comp 0 = x0, 1..3 = x1 d-components, host pre-transposed) -> one big DMA
per slot; output Y mirrors it.  Matmuls run as float32r (same bits,
single-pass PE).  Per slot (tiles [128, W] fp32):

    ACT : x0sq/asq/bsq/csq = squares, A1 = w01*x0 + w00, g0 = u1*x0 + u0,
          B = w04*x0 + w02  (per-partition scale/bias APs)
    POOL: dot = asq + bsq + csq ; out0 = t1 + t2 ; q0 = a*H
    DVE : A = w03*x0sq + A1 ; t1 = x0*A ; t2 = dot*B
          g1 = u2*x0sq + g0 ; G1 = u3*dot + g1
    PE  : graw[h] = gk_s[:, h*128:+128].T @ out0        (h = 0, 1)
    DVE : o0g = (graw0 + gb0) * out0 ; H = (graw1 + gb1) * G1   (fused)
          q1 = b*H ; q2 = c*H
    PE  : y0 = LW0.T @ o0g ; y1_d = LW1.T @ q_d         (LW = lin_w * inv)
    ACT : PSUM -> yout slices, then one DMA out.

Zero padding is safe end-to-end (pad columns produce exact zeros).
"""

import numpy as np

F = 128
S = 64
NCORES = 8
NSLOTS = S // NCORES  # species slots per core
NP0 = 5
NP1 = 4

_PROG_CACHE = {}


def _plan(species):
    """Assign species to (core, slot) and compute padded slot width."""
    counts = np.bincount(species, minlength=S)
    order = np.argsort(-counts, kind="stable")
    core_slots = [[] for _ in range(NCORES)]
    for r in range(NSLOTS):
        cores = range(NCORES) if r % 2 == 0 else range(NCORES - 1, -1, -1)
        for i, c in enumerate(cores):
            core_slots[c].append(int(order[r * NCORES + i]))
    t_seg = max(1, -(-int(counts.max()) // 128))
    return core_slots, 128 * t_seg, counts


def _build_program(W):
    from contextlib import ExitStack

    import concourse.tile as tile
    from concourse import bacc, mybir

    f32 = mybir.dt.float32
    f32r = mybir.dt.float32r
    Alu = mybir.AluOpType
    Act = mybir.ActivationFunctionType
    R = NSLOTS * 4 * W  # fused layout: slot-major, comp, col
    nch = -(-W // 512)  # psum chunks per slot
    PW = 512 * nch      # psum tile width (bank aligned chunks)

    nc = bacc.Bacc(
        "TRN2", target_bir_lowering=False, debug=False, num_devices=NCORES
    )
    X = nc.dram_tensor("X", [F, R], f32, kind="ExternalInput").ap()
    W0T = nc.dram_tensor("W0T", [F, NSLOTS * NP0], f32, kind="ExternalInput").ap()
    W1T = nc.dram_tensor("W1T", [F, NSLOTS * NP1], f32, kind="ExternalInput").ap()
    GK = nc.dram_tensor("GK", [F, NSLOTS * 256], f32r, kind="ExternalInput").ap()
    GB0 = nc.dram_tensor("GB0", [F, NSLOTS], f32, kind="ExternalInput").ap()
    GB1 = nc.dram_tensor("GB1", [F, NSLOTS], f32, kind="ExternalInput").ap()
    LW0 = nc.dram_tensor("LW0", [F, F], f32r, kind="ExternalInput").ap()
    LW1 = nc.dram_tensor("LW1", [F, F], f32r, kind="ExternalInput").ap()
    Y = nc.dram_tensor("Y", [F, R], f32, kind="ExternalOutput").ap()

    def mm(psum_ap, lhsT, rhs, **kw):
        nc.tensor.matmul(psum_ap, lhsT, rhs, **kw)

    with tile.TileContext(nc) as tc:
        with ExitStack() as ctx:
            wp = ctx.enter_context(tc.tile_pool(name="w", bufs=1))
            inp = ctx.enter_context(tc.tile_pool(name="in", bufs=3))
            mid = ctx.enter_context(tc.tile_pool(name="mid", bufs=2))
            sqp = ctx.enter_context(tc.tile_pool(name="sq", bufs=6))
            acp = ctx.enter_context(tc.tile_pool(name="ac", bufs=4))
            ttp = ctx.enter_context(tc.tile_pool(name="tt", bufs=4))
            outp = ctx.enter_context(tc.tile_pool(name="out", bufs=2))
            ps = ctx.enter_context(tc.tile_pool(name="ps", bufs=4, space="PSUM"))

            w0t = wp.tile([F, NSLOTS * NP0], f32)
            w1t = wp.tile([F, NSLOTS * NP1], f32)
            gks = wp.tile([F, NSLOTS * 256], f32r)
            gb0 = wp.tile([F, NSLOTS], f32)
            gb1 = wp.tile([F, NSLOTS], f32)
            lw0 = wp.tile([F, F], f32r)
            lw1 = wp.tile([F, F], f32r)
            nc.sync.dma_start(out=w0t[:], in_=W0T[:])
            nc.sync.dma_start(out=w1t[:], in_=W1T[:])
            nc.sync.dma_start(out=gks[:], in_=GK[:])
            nc.sync.dma_start(out=gb0[:], in_=GB0[:])
            nc.sync.dma_start(out=gb1[:], in_=GB1[:])
            nc.sync.dma_start(out=lw0[:], in_=LW0[:])
            nc.sync.dma_start(out=lw1[:], in_=LW1[:])

            def chunks():
                for c in range(nch):
                    lo = c * 512
                    yield slice(lo, min(W, lo + 512))

            for j in range(NSLOTS):
                xin = inp.tile([F, 4 * W], f32, tag="xin")
                if j < 2:
                    with tc.high_priority():
                        nc.sync.dma_start(
                            out=xin[:], in_=X[:, j * 4 * W : (j + 1) * 4 * W]
                        )
                else:
                    nc.sync.dma_start(
                        out=xin[:], in_=X[:, j * 4 * W : (j + 1) * 4 * W]
                    )
                x0 = xin[:, 0:W]
                xa = xin[:, W : 2 * W]
                xb = xin[:, 2 * W : 3 * W]
                xc = xin[:, 3 * W : 4 * W]

                # per-partition scalar views for this slot's species
                w00 = w0t[:, j * NP0 + 0 : j * NP0 + 1]
                w01 = w0t[:, j * NP0 + 1 : j * NP0 + 2]
                w02 = w0t[:, j * NP0 + 2 : j * NP0 + 3]
                w03 = w0t[:, j * NP0 + 3 : j * NP0 + 4]
                w04 = w0t[:, j * NP0 + 4 : j * NP0 + 5]
                u0 = w1t[:, j * NP1 + 0 : j * NP1 + 1]
                u1 = w1t[:, j * NP1 + 1 : j * NP1 + 2]
                u2 = w1t[:, j * NP1 + 2 : j * NP1 + 3]
                u3 = w1t[:, j * NP1 + 3 : j * NP1 + 4]
                b0 = gb0[:, j : j + 1]
                b1 = gb1[:, j : j + 1]

                x0sq = sqp.tile([F, W], f32, tag="sq", name="x0sq")
                asq = sqp.tile([F, W], f32, tag="sq", name="asq")
                bsq = sqp.tile([F, W], f32, tag="sq", name="bsq")
                csq = sqp.tile([F, W], f32, tag="sq", name="csq")
                nc.scalar.activation(x0sq[:], x0[:], Act.Square)
                nc.scalar.activation(asq[:], xa[:], Act.Square)
                nc.scalar.activation(bsq[:], xb[:], Act.Square)
                nc.scalar.activation(csq[:], xc[:], Act.Square)

                dot = mid.tile([F, W], f32, tag="dot")
                nc.vector.tensor_tensor(dot[:], asq[:], bsq[:], Alu.add)
                nc.vector.tensor_tensor(dot[:], dot[:], csq[:], Alu.add)

                a1 = acp.tile([F, W], f32, tag="ac", name="a1")
                nc.scalar.activation(a1[:], x0[:], Act.Identity, bias=w00, scale=w01)
                aa = mid.tile([F, W], f32, tag="aa")
                nc.vector.scalar_tensor_tensor(
                    aa[:], x0sq[:], w03, a1[:], Alu.mult, Alu.add
                )
                bb = acp.tile([F, W], f32, tag="ac", name="bb")
                nc.scalar.activation(bb[:], x0[:], Act.Identity, bias=w02, scale=w04)

                t1 = ttp.tile([F, W], f32, tag="tt", name="t1")
                nc.vector.tensor_tensor(t1[:], x0[:], aa[:], Alu.mult)
                t2 = ttp.tile([F, W], f32, tag="tt", name="t2")
                nc.vector.tensor_tensor(t2[:], dot[:], bb[:], Alu.mult)
                out0 = mid.tile([F, W], f32r, tag="out0")
                nc.vector.tensor_tensor(out0[:], t1[:], t2[:], Alu.add)

                # gate matmuls: graw[h] = gk[:, h-half].T @ out0
                praw0 = ps.tile([F, PW], f32, tag="ps")
                praw1 = ps.tile([F, PW], f32, tag="ps")
                for h, pr in ((0, praw0), (1, praw1)):
                    lhsT = gks[:, j * 256 + h * 128 : j * 256 + (h + 1) * 128]
                    for cs in chunks():
                        mm(pr[:, cs], lhsT, out0[:, cs], start=True, stop=True)

                g0 = acp.tile([F, W], f32, tag="ac", name="g0")
                nc.scalar.activation(g0[:], x0[:], Act.Identity, bias=u0, scale=u1)
                g1 = ttp.tile([F, W], f32, tag="tt", name="g1")
                nc.vector.scalar_tensor_tensor(
                    g1[:], x0sq[:], u2, g0[:], Alu.mult, Alu.add
                )
                gg = mid.tile([F, W], f32, tag="gg")
                nc.vector.scalar_tensor_tensor(
                    gg[:], dot[:], u3, g1[:], Alu.mult, Alu.add
                )

                o0g = mid.tile([F, W], f32r, tag="o0g")
                nc.vector.scalar_tensor_tensor(
                    o0g[:], praw0[:, :W], b0, out0[:], Alu.add, Alu.mult
                )
                hh = mid.tile([F, W], f32, tag="hh")
                nc.vector.scalar_tensor_tensor(
                    hh[:], praw1[:, :W], b1, gg[:], Alu.add, Alu.mult
                )

                q0 = mid.tile([F, W], f32r, tag="q0")
                nc.vector.tensor_tensor(q0[:], xa[:], hh[:], Alu.mult)
                q1 = mid.tile([F, W], f32r, tag="q1")
                nc.vector.tensor_tensor(q1[:], xb[:], hh[:], Alu.mult)
                q2 = mid.tile([F, W], f32r, tag="q2")
                nc.vector.tensor_tensor(q2[:], xc[:], hh[:], Alu.mult)

                py = [
                    ps.tile([F, PW], f32, tag="ps", name=f"py{i}") for i in range(4)
                ]
                for t, rhs, lhsT in (
                    (py[0], o0g, lw0),
                    (py[1], q0, lw1),
                    (py[2], q1, lw1),
                    (py[3], q2, lw1),
                ):
                    for cs in chunks():
                        mm(t[:, cs], lhsT[:], rhs[:, cs], start=True, stop=True)

                yout = outp.tile([F, 4 * W], f32, tag="yout")
                for comp in range(4):
                    nc.scalar.copy(
                        yout[:, comp * W : (comp + 1) * W], py[comp][:, :W]
                    )
                nc.sync.dma_start(
                    out=Y[:, j * 4 * W : (j + 1) * 4 * W], in_=yout[:]
                )

    nc.compile()
    return nc


def _get_program(W):
    if W not in _PROG_CACHE:
        _PROG_CACHE[W] = _build_program(W)
    return _PROG_CACHE[W]


def _prepare(node_feats, node_species, w0, w1, gate_kernel, gate_bias, lin_w0, lin_w1):
    N = node_feats.shape[0]
    species = np.asarray(node_species).astype(np.int64)
    core_slots, W, counts = _plan(species)
    R = NSLOTS * 4 * W

    perm = np.argsort(species, kind="stable")
    starts = np.zeros(S + 1, np.int64)
    starts[1:] = np.cumsum(counts)

    nf = np.ascontiguousarray(np.asarray(node_feats, dtype=np.float32))
    x0t = nf[:, :F].T  # [F, N]
    x1 = nf[:, F:].reshape(N, F, 3)
    xt = [x0t, x1[:, :, 0].T, x1[:, :, 1].T, x1[:, :, 2].T]

    inv = np.float32(1.0 / np.sqrt(F))
    lw0 = np.ascontiguousarray(np.asarray(lin_w0, np.float32) * inv)
    lw1 = np.ascontiguousarray(np.asarray(lin_w1, np.float32) * inv)
    w0 = np.asarray(w0, np.float32)
    w1 = np.asarray(w1, np.float32)
    gk = np.asarray(gate_kernel, np.float32)
    gb = np.asarray(gate_bias, np.float32)

    in_maps = []
    gathers = []  # per core: (src_node_idx, per-comp dst cols)
    for c in range(NCORES):
        slots = core_slots[c]
        src = []
        base = []  # col position of each node within its slot (j*4W + col)
        for j, s in enumerate(slots):
            n_s = int(counts[s])
            if n_s:
                src.append(perm[starts[s] : starts[s] + n_s])
                base.append(np.arange(n_s, dtype=np.int64) + j * 4 * W)
        src = np.concatenate(src) if src else np.zeros(0, np.int64)
        base = np.concatenate(base) if base else np.zeros(0, np.int64)
        gathers.append((src, base))

        Xc = np.zeros((F, R), np.float32)
        for comp in range(4):
            Xc[:, base + comp * W] = xt[comp][:, src]

        sw0 = w0[slots]  # [8, 5, F]
        sw1 = w1[slots]
        in_maps.append(
            dict(
                X=Xc,
                W0T=np.ascontiguousarray(sw0.transpose(2, 0, 1).reshape(F, -1)),
                W1T=np.ascontiguousarray(sw1.transpose(2, 0, 1).reshape(F, -1)),
                GK=np.ascontiguousarray(gk[slots].transpose(1, 0, 2).reshape(F, -1)),
                GB0=np.ascontiguousarray(gb[slots][:, :F].T),
                GB1=np.ascontiguousarray(gb[slots][:, F:].T),
                LW0=lw0,
                LW1=lw1,
            )
        )
    return in_maps, gathers, W, N


def _assemble(results, gathers, W, N):
    out = np.empty((N, 4 * F), np.float32)
    y0t = np.empty((F, N), np.float32)
    y1t = np.empty((3, F, N), np.float32)
    for c in range(NCORES):
        Yc = results[c]["Y"]
        src, base = gathers[c]
        y0t[:, src] = Yc[:, base]
        for d in range(3):
            y1t[d][:, src] = Yc[:, base + (1 + d) * W]
    out[:, :F] = y0t.T
    out[:, F:] = y1t.transpose(2, 1, 0).reshape(N, 3 * F)
    return out


def kernel(**inputs):
    from concourse.bass_utils import run_bass_kernel_spmd

    in_maps, gathers, W, N = _prepare(**inputs)
    nc = _get_program(W)
    res = run_bass_kernel_spmd(nc, in_maps, list(range(NCORES)))
    return _assemble(res.results, gathers, W, N)


# revision 13
# speedup vs baseline: 1.3975x; 1.0263x over previous
"""Trainium2 Bass kernel for EquivariantProductBasisBlock.

Strategy
--------
The per-node compute is feature-diagonal except for three matmuls that
contract over the feature axis (species-conditioned gate + two equivariant
linears).  We therefore keep F=128 on SBUF partitions and nodes on the free
axis, so every matmul is a natural PE op and every elementwise op is a
[128, W] streaming op.

The species-conditioned weights force species-uniform node tiles, so the
host sorts nodes by species and assigns 8 species per core (snake order on
descending counts for load balance).  Each core's program processes 8
"slots" (species segments) of identical padded width W = 128 * ceil(max
species count / 128) -- the program is identical across cores (SPMD); only
the data (which species lives in which slot) differs.

Device data layout: one fused input array X[f, slot*(4W) + comp*W + col]
(comp 0 = x0, 1..3 = x1 d-components, host pre-transposed) -> one big DMA
per slot; output Y mirrors it.  Matmuls run as float32r (same bits,
single-pass PE).  Per slot (tiles [128, W] fp32):

    ACT : x0sq/asq/bsq/csq = squares, A1 = w01*x0 + w00, g0 = u1*x0 + u0,
          B = w04*x0 + w02  (per-partition scale/bias APs)
    POOL: dot = asq + bsq + csq ; out0 = t1 + t2 ; q0 = a*H
    DVE : A = w03*x0sq + A1 ; t1 = x0*A ; t2 = dot*B
          g1 = u2*x0sq + g0 ; G1 = u3*dot + g1
    PE  : graw[h] = gk_s[:, h*128:+128].T @ out0        (h = 0, 1)
    DVE : o0g = (graw0 + gb0) * out0 ; H = (graw1 + gb1) * G1   (fused)
          q1 = b*H ; q2 = c*H
    PE  : y0 = LW0.T @ o0g ; y1_d = LW1.T @ q_d         (LW = lin_w * inv)
    ACT : PSUM -> yout slices, then one DMA out.

Zero padding is safe end-to-end (pad columns produce exact zeros).
"""

import numpy as np

F = 128
S = 64
NCORES = 8
NSLOTS = S // NCORES  # species slots per core
NP0 = 5
NP1 = 4

_PROG_CACHE = {}


def _plan(species):
    """Assign species to (core, slot) and compute padded slot width."""
    counts = np.bincount(species, minlength=S)
    order = np.argsort(-counts, kind="stable")
    core_slots = [[] for _ in range(NCORES)]
    for r in range(NSLOTS):
        cores = range(NCORES) if r % 2 == 0 else range(NCORES - 1, -1, -1)
        for i, c in enumerate(cores):
            core_slots[c].append(int(order[r * NCORES + i]))
    t_seg = max(1, -(-int(counts.max()) // 128))
    return core_slots, 128 * t_seg, counts


def _build_program(W):
    from contextlib import ExitStack

    import concourse.tile as tile
    from concourse import bacc, mybir

    f32 = mybir.dt.float32
    f32r = mybir.dt.float32r
    f16 = mybir.dt.float16
    Alu = mybir.AluOpType
    Act = mybir.ActivationFunctionType
    R = NSLOTS * 4 * W  # fused layout: slot-major, comp, col
    nch = -(-W // 512)  # psum chunks per slot
    PW = 512 * nch      # psum tile width (bank aligned chunks)

    nc = bacc.Bacc(
        "TRN2", target_bir_lowering=False, debug=False, num_devices=NCORES
    )
    X = nc.dram_tensor("X", [F, R], f32, kind="ExternalInput").ap()
    X16 = nc.dram_tensor("X16", [F, NSLOTS * 3 * W], f16, kind="ExternalInput").ap()
    W0T = nc.dram_tensor("W0T", [F, NSLOTS * NP0], f32, kind="ExternalInput").ap()
    W1T = nc.dram_tensor("W1T", [F, NSLOTS * NP1], f32, kind="ExternalInput").ap()
    GK = nc.dram_tensor("GK", [F, NSLOTS * 256], f32r, kind="ExternalInput").ap()
    GB0 = nc.dram_tensor("GB0", [F, NSLOTS], f32, kind="ExternalInput").ap()
    GB1 = nc.dram_tensor("GB1", [F, NSLOTS], f32, kind="ExternalInput").ap()
    LW0 = nc.dram_tensor("LW0", [F, F], f32r, kind="ExternalInput").ap()
    LW1 = nc.dram_tensor("LW1", [F, F], f16, kind="ExternalInput").ap()
    Y = nc.dram_tensor("Y", [F, R], f32, kind="ExternalOutput").ap()

    def mm(psum_ap, lhsT, rhs, **kw):
        nc.tensor.matmul(psum_ap, lhsT, rhs, **kw)

    with tile.TileContext(nc) as tc:
        with ExitStack() as ctx:
            wp = ctx.enter_context(tc.tile_pool(name="w", bufs=1))
            inp = ctx.enter_context(tc.tile_pool(name="in", bufs=2))
            mid = ctx.enter_context(tc.tile_pool(name="mid", bufs=2))
            sqp = ctx.enter_context(tc.tile_pool(name="sq", bufs=6))
            acp = ctx.enter_context(tc.tile_pool(name="ac", bufs=4))
            ttp = ctx.enter_context(tc.tile_pool(name="tt", bufs=4))
            outp = ctx.enter_context(tc.tile_pool(name="out", bufs=2))
            ps = ctx.enter_context(tc.tile_pool(name="ps", bufs=4, space="PSUM"))

            w0t = wp.tile([F, NSLOTS * NP0], f32)
            w1t = wp.tile([F, NSLOTS * NP1], f32)
            gks = wp.tile([F, NSLOTS * 256], f32r)
            gb0 = wp.tile([F, NSLOTS], f32)
            gb1 = wp.tile([F, NSLOTS], f32)
            lw0 = wp.tile([F, F], f32r)
            lw1 = wp.tile([F, F], f16)
            nc.sync.dma_start(out=w0t[:], in_=W0T[:])
            nc.sync.dma_start(out=w1t[:], in_=W1T[:])
            nc.sync.dma_start(out=gks[:], in_=GK[:])
            nc.sync.dma_start(out=gb0[:], in_=GB0[:])
            nc.sync.dma_start(out=gb1[:], in_=GB1[:])
            nc.sync.dma_start(out=lw0[:], in_=LW0[:])
            nc.sync.dma_start(out=lw1[:], in_=LW1[:])

            def chunks():
                for c in range(nch):
                    lo = c * 512
                    yield slice(lo, min(W, lo + 512))

            for j in range(NSLOTS):
                xin = inp.tile([F, 4 * W], f32, tag="xin")
                x16 = inp.tile([F, 3 * W], f16, tag="x16")

                def load_inputs():
                    nc.sync.dma_start(
                        out=xin[:, W : 4 * W],
                        in_=X[:, j * 4 * W + W : (j + 1) * 4 * W],
                    )
                    nc.sync.dma_start(
                        out=xin[:, 0:W], in_=X[:, j * 4 * W : j * 4 * W + W]
                    )
                    nc.sync.dma_start(
                        out=x16[:], in_=X16[:, j * 3 * W : (j + 1) * 3 * W]
                    )

                if j < 2:
                    with tc.high_priority():
                        load_inputs()
                else:
                    load_inputs()
                x0 = xin[:, 0:W]
                xa = xin[:, W : 2 * W]
                xb = xin[:, 2 * W : 3 * W]
                xc = xin[:, 3 * W : 4 * W]

                # per-partition scalar views for this slot's species
                w00 = w0t[:, j * NP0 + 0 : j * NP0 + 1]
                w01 = w0t[:, j * NP0 + 1 : j * NP0 + 2]
                w02 = w0t[:, j * NP0 + 2 : j * NP0 + 3]
                w03 = w0t[:, j * NP0 + 3 : j * NP0 + 4]
                w04 = w0t[:, j * NP0 + 4 : j * NP0 + 5]
                u0 = w1t[:, j * NP1 + 0 : j * NP1 + 1]
                u1 = w1t[:, j * NP1 + 1 : j * NP1 + 2]
                u2 = w1t[:, j * NP1 + 2 : j * NP1 + 3]
                u3 = w1t[:, j * NP1 + 3 : j * NP1 + 4]
                b0 = gb0[:, j : j + 1]
                b1 = gb1[:, j : j + 1]

                sqc = sqp.tile([F, 3 * W], f32, tag="sqc", name="sqc", bufs=2)
                nc.scalar.activation(sqc[:], xin[:, W : 4 * W], Act.Square)
                x0sq = sqp.tile([F, W], f32, tag="x0sq", name="x0sq", bufs=2)
                nc.scalar.activation(x0sq[:], x0[:], Act.Square)

                dot = mid.tile([F, W], f32, tag="dot")
                nc.vector.tensor_tensor(
                    dot[:], sqc[:, 0:W], sqc[:, W : 2 * W], Alu.add
                )
                nc.vector.tensor_tensor(
                    dot[:], dot[:], sqc[:, 2 * W : 3 * W], Alu.add
                )

                a1 = acp.tile([F, W], f32, tag="ac", name="a1")
                nc.scalar.activation(a1[:], x0[:], Act.Identity, bias=w00, scale=w01)
                aa = mid.tile([F, W], f32, tag="aa")
                nc.vector.scalar_tensor_tensor(
                    aa[:], x0sq[:], w03, a1[:], Alu.mult, Alu.add
                )
                bb = acp.tile([F, W], f32, tag="ac", name="bb")
                nc.scalar.activation(bb[:], x0[:], Act.Identity, bias=w02, scale=w04)

                t1 = ttp.tile([F, W], f32, tag="tt", name="t1")
                nc.vector.tensor_tensor(t1[:], x0[:], aa[:], Alu.mult)
                t2 = ttp.tile([F, W], f32, tag="tt", name="t2")
                nc.vector.tensor_tensor(t2[:], dot[:], bb[:], Alu.mult)
                out0 = mid.tile([F, W], f32r, tag="out0")
                nc.vector.tensor_tensor(out0[:], t1[:], t2[:], Alu.add)

                # gate matmuls: graw[h] = gk[:, h-half].T @ out0
                praw0 = ps.tile([F, PW], f32, tag="ps")
                praw1 = ps.tile([F, PW], f32, tag="ps")
                for h, pr in ((0, praw0), (1, praw1)):
                    lhsT = gks[:, j * 256 + h * 128 : j * 256 + (h + 1) * 128]
                    for cs in chunks():
                        mm(pr[:, cs], lhsT, out0[:, cs], start=True, stop=True)

                g0 = acp.tile([F, W], f32, tag="ac", name="g0")
                nc.scalar.activation(g0[:], x0[:], Act.Identity, bias=u0, scale=u1)
                g1 = ttp.tile([F, W], f32, tag="tt", name="g1")
                nc.vector.scalar_tensor_tensor(
                    g1[:], x0sq[:], u2, g0[:], Alu.mult, Alu.add
                )
                gg = mid.tile([F, W], f32, tag="gg")
                nc.vector.scalar_tensor_tensor(
                    gg[:], dot[:], u3, g1[:], Alu.mult, Alu.add
                )

                o0g = mid.tile([F, W], f32r, tag="o0g")
                nc.vector.scalar_tensor_tensor(
                    o0g[:], praw0[:, :W], b0, out0[:], Alu.add, Alu.mult
                )
                hh = mid.tile([F, W], f16, tag="hh")
                nc.vector.scalar_tensor_tensor(
                    hh[:], praw1[:, :W], b1, gg[:], Alu.add, Alu.mult
                )

                qc = mid.tile([F, 3 * W], f16, tag="qc")
                hh3 = hh[:].unsqueeze(1).broadcast_to([F, 3, W])
                nc.vector.tensor_tensor(
                    qc[:].rearrange("p (d w) -> p d w", d=3), x16[:].rearrange(
                        "p (d w) -> p d w", d=3
                    ), hh3, Alu.mult
                )
                q0 = qc[:, 0:W]
                q1 = qc[:, W : 2 * W]
                q2 = qc[:, 2 * W : 3 * W]

                py = [
                    ps.tile([F, PW], f32, tag="ps", name=f"py{i}") for i in range(4)
                ]
                for t, rhs, lhsT in (
                    (py[0], o0g, lw0),
                    (py[1], q0, lw1),
                    (py[2], q1, lw1),
                    (py[3], q2, lw1),
                ):
                    for cs in chunks():
                        mm(t[:, cs], lhsT[:], rhs[:, cs], start=True, stop=True)

                yout = outp.tile([F, 4 * W], f32, tag="yout")
                for comp in range(4):
                    nc.scalar.copy(
                        yout[:, comp * W : (comp + 1) * W], py[comp][:, :W]
                    )
                nc.sync.dma_start(
                    out=Y[:, j * 4 * W : (j + 1) * 4 * W], in_=yout[:]
                )

    nc.compile()
    return nc


def _get_program(W):
    if W not in _PROG_CACHE:
        _PROG_CACHE[W] = _build_program(W)
    return _PROG_CACHE[W]


def _prepare(node_feats, node_species, w0, w1, gate_kernel, gate_bias, lin_w0, lin_w1):
    N = node_feats.shape[0]
    species = np.asarray(node_species).astype(np.int64)
    core_slots, W, counts = _plan(species)
    R = NSLOTS * 4 * W

    perm = np.argsort(species, kind="stable")
    starts = np.zeros(S + 1, np.int64)
    starts[1:] = np.cumsum(counts)

    nf = np.ascontiguousarray(np.asarray(node_feats, dtype=np.float32))
    x0t = nf[:, :F].T  # [F, N]
    x1 = nf[:, F:].reshape(N, F, 3)
    xt = [x0t, x1[:, :, 0].T, x1[:, :, 1].T, x1[:, :, 2].T]

    inv = np.float32(1.0 / np.sqrt(F))
    lw0 = np.ascontiguousarray(np.asarray(lin_w0, np.float32) * inv)
    lw1 = np.ascontiguousarray((np.asarray(lin_w1, np.float32) * inv).astype(np.float16))
    w0 = np.asarray(w0, np.float32)
    w1 = np.asarray(w1, np.float32)
    gk = np.asarray(gate_kernel, np.float32)
    gb = np.asarray(gate_bias, np.float32)

    in_maps = []
    gathers = []  # per core: (src_node_idx, per-comp dst cols)
    for c in range(NCORES):
        slots = core_slots[c]
        src = []
        base = []  # col position of each node within its slot (j*4W + col)
        for j, s in enumerate(slots):
            n_s = int(counts[s])
            if n_s:
                src.append(perm[starts[s] : starts[s] + n_s])
                base.append(np.arange(n_s, dtype=np.int64) + j * 4 * W)
        src = np.concatenate(src) if src else np.zeros(0, np.int64)
        base = np.concatenate(base) if base else np.zeros(0, np.int64)
        gathers.append((src, base))

        Xc = np.zeros((F, R), np.float32)
        for comp in range(4):
            Xc[:, base + comp * W] = xt[comp][:, src]
        # fp16 copy of the x1 block, per-slot layout [3W per slot]
        X16c = np.zeros((F, NSLOTS * 3 * W), np.float16)
        base16 = base - (base // (4 * W)) * W  # j*4W+col -> j*3W+col
        for comp in range(3):
            X16c[:, base16 + comp * W] = xt[1 + comp][:, src].astype(np.float16)

        sw0 = w0[slots]  # [8, 5, F]
        sw1 = w1[slots]
        in_maps.append(
            dict(
                X=Xc,
                X16=X16c,
                W0T=np.ascontiguousarray(sw0.transpose(2, 0, 1).reshape(F, -1)),
                W1T=np.ascontiguousarray(sw1.transpose(2, 0, 1).reshape(F, -1)),
                GK=np.ascontiguousarray(gk[slots].transpose(1, 0, 2).reshape(F, -1)),
                GB0=np.ascontiguousarray(gb[slots][:, :F].T),
                GB1=np.ascontiguousarray(gb[slots][:, F:].T),
                LW0=lw0,
                LW1=lw1,
            )
        )
    return in_maps, gathers, W, N


def _assemble(results, gathers, W, N):
    out = np.empty((N, 4 * F), np.float32)
    y0t = np.empty((F, N), np.float32)
    y1t = np.empty((3, F, N), np.float32)
    for c in range(NCORES):
        Yc = results[c]["Y"]
        src, base = gathers[c]
        y0t[:, src] = Yc[:, base]
        for d in range(3):
            y1t[d][:, src] = Yc[:, base + (1 + d) * W]
    out[:, :F] = y0t.T
    out[:, F:] = y1t.transpose(2, 1, 0).reshape(N, 3 * F)
    return out


def kernel(**inputs):
    from concourse.bass_utils import run_bass_kernel_spmd

    in_maps, gathers, W, N = _prepare(**inputs)
    nc = _get_program(W)
    res = run_bass_kernel_spmd(nc, in_maps, list(range(NCORES)))
    return _assemble(res.results, gathers, W, N)
